# revision 6
# baseline (speedup 1.0000x reference)
"""Self-contained Trainium2 Bass kernel for the CharRNN problem:
2-layer LSTM (B=32, T=256, H=256) + V=32000 softmax cross-entropy mean loss.

Strategy (8 NeuronCores, SPMD):
  * the LSTM recurrence is replicated on every core (latency-bound)
  * the softmax matmul + exp is sharded over the vocab: each core owns a
    4000-wide shard of softmax_w, computes logits for all 8192 rows against
    its shard, reduces them to per-row sum(exp(logit)) plus the per-row
    target logit; the host combines loss_r = log(sum_c se_r) - tgt_logit_r

Device-side structure (v3 — chain-optimized):
  * wavefront: slot t runs L1 step t and L2 step t-1 so the two layer
    recurrence chains interleave on the engines
  * h transposes via DVE 32x32 StreamTranspose (2 blocks per op,
    cross-partition writes straight into the hidden-major slabs)
  * gate column order [i, o, j, f] with the 0.5 sigmoid input scale folded
    into W on the host; the forget-gate +0.5 bias is added in PSUM by a
    K=1 ones-row matmul -> ONE tanh ACT call per layer-step
  * cell-update add and the target-logit elementwise multiply run on
    GpSimd (the Vector engine is the busiest)
  * exp over PAIRS of 500-wide vocab chunks ([128,2,500] strided AP);
    per slot the logits MMs are emitted FIRST (PE fill work while the
    recurrence chain runs) and the exp ACT call is emitted BETWEEN the two
    layer tails, filling the ACT gap
"""
import numpy as np
import ml_dtypes
import concourse.bass as bass
import concourse.mybir as mybir
import concourse.tile as tile
from concourse import bacc
from concourse.bass_utils import run_bass_kernel_spmd

F32 = mybir.dt.float32
BF16 = mybir.dt.bfloat16
I32 = mybir.dt.int32
I16 = mybir.dt.int16
AF = mybir.ActivationFunctionType
ALU = mybir.AluOpType

B, T, H, V, NCORES = 32, 256, 256, 32000, 8


def build_charrnn(T=256, V=32000, n_cores=8, has_b1=False, has_b2=False,
                  has_swb=False, num_devices=8):
    B, H = 32, 256
    G4 = 4 * H                      # 1024 gate width
    VS = V // n_cores               # vocab shard per core
    BT = B * T
    RT = BT // 128                  # 128-row tiles (4 steps each)
    assert T % 4 == 0 and BT % 128 == 0

    # one psum BANK per matmul chunk (a matmul may not cross a bank)
    CH = max(d for d in range(1, 513) if VS % d == 0)   # 500
    NCHUNK = VS // CH                                    # 8
    NPAIR = NCHUNK // 2                                  # 4 exp calls per tile

    nc = bacc.Bacc("TRN2", target_bir_lowering=False, debug=False,
                   num_devices=num_devices)

    # ---------------- DRAM I/O ----------------
    ids_d = nc.dram_tensor("ids", (RT, 128, 1), I32, kind="ExternalInput")
    emb_d = nc.dram_tensor("emb", (V, H), F32, kind="ExternalInput")
    w1_d = nc.dram_tensor("w1", (4, 128, G4), BF16, kind="ExternalInput")
    w2_d = nc.dram_tensor("w2", (4, 128, G4), BF16, kind="ExternalInput")
    sw_d = nc.dram_tensor("sw", (2, 128, VS), BF16, kind="ExternalInput")
    swp_d = nc.dram_tensor("swp", (2, 128, VS, 2), I16, kind="ExternalInput")
    tgi_d = nc.dram_tensor("tgi", (RT, 128, 8), I16, kind="ExternalInput")
    if has_b1:
        b1_d = nc.dram_tensor("b1p", (32, G4), F32, kind="ExternalInput")
    if has_b2:
        b2_d = nc.dram_tensor("b2p", (32, G4), F32, kind="ExternalInput")
    if has_swb:
        swb_d = nc.dram_tensor("swbp", (128, VS), F32, kind="ExternalInput")
    se_d = nc.dram_tensor("se_out", (128, RT * NPAIR), F32,
                          kind="ExternalOutput")
    tg_d = nc.dram_tensor("tg_out", (1, BT), F32, kind="ExternalOutput")

    with tile.TileContext(nc) as tc:
        with tc.tile_pool(name="persist", bufs=1) as pp:
            # ---- persistent SBUF ----
            w1_sb = pp.tile([128, 4, G4], BF16, tag="w1")
            w2_sb = pp.tile([128, 4, G4], BF16, tag="w2")
            nc.sync.dma_start(w1_sb[:], w1_d[:].rearrange("k p c -> p k c"))
            nc.sync.dma_start(w2_sb[:], w2_d[:].rearrange("k p c -> p k c"))
            sw_sb = pp.tile([128, 2, VS], BF16, tag="sw")
            nc.sync.dma_start(sw_sb[:], sw_d[:].rearrange("k p c -> p k c"))
            swp_sb = pp.tile([128, 2, VS, 2], I16, tag="swp")
            nc.sync.dma_start(swp_sb[:],
                              swp_d[:].rearrange("k p c d -> p k c d"))
            hs = pp.tile([128, 2, BT], BF16, tag="hs")

            ones_bf = pp.tile([128, 1], BF16, tag="ones")
            nc.gpsimd.memset(ones_bf[:], 1.0)
            # forget-gate bias row: z_f += 0.5 (post W-fold) via a K=1
            # matmul closing the half-1 accumulation group
            fb_row = pp.tile([1, 512], BF16, tag="fbrow")
            nc.gpsimd.memset(fb_row[:, 0:256], 0.0)
            nc.gpsimd.memset(fb_row[:, 256:512], 0.5)

            c1 = pp.tile([32, H], F32, tag="c1")
            c2 = pp.tile([32, H], F32, tag="c2")
            nc.gpsimd.memset(c1[:], 0.0)
            nc.gpsimd.memset(c2[:], 0.0)

            se_sb = pp.tile([128, RT * NPAIR], F32, tag="se")
            tg_sb = pp.tile([1, BT], F32, tag="tg")
            # accum_out adds into existing SBUF content on HW — zero it
            nc.gpsimd.memset(se_sb[:], 0.0)

            if has_b1:
                b1_sb = pp.tile([32, G4], F32, tag="b1")
                nc.sync.dma_start(b1_sb[:], b1_d[:])
            if has_b2:
                b2_sb = pp.tile([32, G4], F32, tag="b2")
                nc.sync.dma_start(b2_sb[:], b2_d[:])
            if has_swb:
                swb_sb = pp.tile([128, VS], F32, tag="swb")
                nc.sync.dma_start(swb_sb[:], swb_d[:])

            # ============ fused phase: gather + LSTM + logits ============
            with (
                tc.tile_pool(name="xsp", bufs=1) as xsp,
                tc.tile_pool(name="stage", bufs=3) as stp,
                tc.tile_pool(name="lwork", bufs=3) as lw,
                tc.tile_pool(name="zp", bufs=2, space="PSUM") as zp,
                tc.tile_pool(name="ep", bufs=2, space="PSUM") as ep,
                tc.tile_pool(name="ework", bufs=3) as ew,
            ):
                xs = xsp.tile([128, 2, BT], BF16, tag="xs")

                # ---- embedding gather (time-major) + transpose to slabs ----
                for rt in range(RT):
                    ids_sb = stp.tile([128, 1], I32, tag="ids")
                    nc.gpsimd.dma_start(ids_sb[:], ids_d.ap()[rt])
                    xrow = stp.tile([128, H], F32, tag="xrow")
                    nc.gpsimd.indirect_dma_start(
                        out=xrow[:], out_offset=None,
                        in_=emb_d[:],
                        in_offset=bass.IndirectOffsetOnAxis(
                            ap=ids_sb[:, :1], axis=0),
                    )
                    xbf = stp.tile([128, H], BF16, tag="xbf")
                    nc.scalar.copy(xbf[:], xrow[:])
                    cs = 128 * rt
                    nc.sync.dma_start_transpose(
                        xs[:, 0, cs:cs + 128], xbf[:, 0:128])
                    nc.sync.dma_start_transpose(
                        xs[:, 1, cs:cs + 128], xbf[:, 128:256])

                def emit_logits_mms(rt, p):
                    """Logits matmuls for vocab chunks (2p, 2p+1) of row-tile
                    rt; p==3 also emits the target-logit gather+reduce.
                    Returns state for the deferred exp/copy emission."""
                    cs = 128 * rt
                    pse = ep.tile([128, 2, 512], F32, tag="pse")
                    for half, c in enumerate((2 * p, 2 * p + 1)):
                        for k in range(2):
                            nc.tensor.matmul(
                                pse[:, half, 0:CH], hs[:, k, cs:cs + 128],
                                sw_sb[:, k, c * CH:c * CH + CH],
                                start=(k == 0), stop=(k == 1),
                            )
                        if has_swb:
                            nc.vector.tensor_tensor(
                                out=pse[:, half, 0:CH], in0=pse[:, half, 0:CH],
                                in1=swb_sb[:, (2 * p + half) * CH:
                                           (2 * p + half) * CH + CH],
                                op=ALU.add)
                    pst = None
                    if p == 3:
                        tgi_sb = ew.tile([128, 8], I16, tag="tgi")
                        nc.gpsimd.dma_start(tgi_sb[:], tgi_d.ap()[rt])
                        pstt = ep.tile([128, 2, 512], F32, tag="pse")
                        pst = pstt[0:1, 0, 0:128]
                        for k in range(2):
                            swg = ew.tile([128, 128, 2], I16, tag="swg")
                            nc.gpsimd.ap_gather(
                                swg[:], swp_sb[:, k], tgi_sb[:],
                                channels=128, num_elems=VS, d=2, num_idxs=128,
                            )
                            mulk = ew.tile([128, 128], BF16, tag="mulk")
                            nc.gpsimd.tensor_tensor(
                                out=mulk[:],
                                in0=swg[:].bitcast(BF16)[:, :, 0],
                                in1=hs[:, k, cs:cs + 128],
                                op=ALU.mult)
                            nc.tensor.matmul(pst, ones_bf[:, 0:1], mulk[:],
                                             start=(k == 0), stop=(k == 1))
                    return pse, pst, rt, p, cs

                def emit_exp(state):
                    pse, pst, rt, p, cs = state
                    ebuf = ew.tile([128, 2, CH], BF16, tag="ebuf")
                    nc.scalar.activation(
                        ebuf[:], pse[:, :, 0:CH], AF.Exp,
                        accum_out=se_sb[:, rt * NPAIR + p:rt * NPAIR + p + 1])
                    if pst is not None:
                        nc.scalar.copy(tg_sb[0:1, cs:cs + 128], pst)

                def emit_gate_mms(psz, lhsTs, w_sb, k0, start, stop):
                    """k-tile matmuls into the [32,1024] gate psum; when
                    `stop`, the half-1 (j,f) group is closed by the K=1
                    forget-bias matmul instead of the last k-tile."""
                    for ki, lt in enumerate(lhsTs):
                        k = k0 + ki
                        first = start and k == 0
                        last = stop and ki == len(lhsTs) - 1
                        nc.tensor.matmul(
                            psz[:, 0:512], lt, w_sb[:, k, 0:512],
                            start=first, stop=last)
                        nc.tensor.matmul(
                            psz[:, 512:1024], lt, w_sb[:, k, 512:1024],
                            start=first, stop=False)
                    if stop:
                        nc.tensor.matmul(
                            psz[:, 512:1024], fb_ones[0:1, :], fb_row[0:1, :],
                            start=False, stop=True)

                fb_ones = pp.tile([1, 32], BF16, tag="fbones")
                nc.gpsimd.memset(fb_ones[:], 1.0)

                def lstm_tail(psz, c_sb, bias_sb):
                    """Gate activations + cell update. Gate col order
                    [i, o, j, f]; sigmoid input scales pre-folded into W and
                    the f +0.5 bias added in psum, so ONE plain tanh covers
                    all gates (sigmoid(x) = 0.5*tanh(x/2) + 0.5; the outer
                    affine is applied by affine_mul_reduce)."""
                    if bias_sb is not None:
                        nc.vector.tensor_tensor(
                            out=psz[:], in0=psz[:], in1=bias_sb[:],
                            op=ALU.add)
                    g = lw.tile([32, G4], BF16, tag="g")
                    nc.scalar.activation(g[:], psz[:], AF.Tanh)
                    junk = lw.tile([32, 1], F32, tag="junk")
                    t1 = lw.tile([32, H], F32, tag="t1")
                    nc.vector.affine_mul_reduce(
                        t1[:], junk[:], g[:, 0:256], g[:, 512:768], 0.5, 0.5)
                    cf = lw.tile([32, H], F32, tag="cf")
                    nc.vector.affine_mul_reduce(
                        cf[:], junk[:], g[:, 768:1024], c_sb[:], 0.5, 0.5)
                    nc.gpsimd.tensor_tensor(out=c_sb[:], in0=cf[:],
                                            in1=t1[:], op=ALU.add)
                    tc_t = lw.tile([32, H], BF16, tag="tc")
                    nc.scalar.activation(tc_t[:], c_sb[:], AF.Tanh)
                    hrow = lw.tile([32, H], BF16, tag="hrow")
                    nc.vector.affine_mul_reduce(
                        hrow[:], junk[:], g[:, 256:512], tc_t[:], 0.5, 0.5)
                    return hrow

                def transpose_to(hrow, dst):
                    """hrow [32,256] -> dst [128,2,32] hidden-major k-tiles
                    via DVE StreamTranspose, 2 blocks per op."""
                    hv = hrow[:].rearrange("p (k q b) -> p k q b",
                                           k=2, q=4, b=32)
                    for q in range(4):
                        nc.vector.transpose(
                            dst[32 * q:32 * q + 32], hv[:, :, q, :])

                # ---- wavefront: slot t = L1 step t  +  L2 step t-1 ----
                h1T_prev = None
                for t in range(T + 1):
                    ei = t - 6
                    h1T_tm1 = h1T_prev
                    psz1 = psz2 = None

                    # L1(t) x-part: no dependency on the recurrence
                    if t < T:
                        ts0 = 32 * t
                        psz1 = zp.tile([32, G4], F32, tag="z")
                        emit_gate_mms(
                            psz1,
                            [xs[:, 0, ts0:ts0 + 32], xs[:, 1, ts0:ts0 + 32]],
                            w1_sb, 0, start=True, stop=False)
                    # PE fill while the chain runs
                    estate = None
                    if ei >= 0:
                        estate = emit_logits_mms(ei // 4, ei % 4)
                    # L1(t) h-part (waits on h1T(t-1))
                    if t < T:
                        if h1T_tm1 is not None:
                            emit_gate_mms(
                                psz1, [h1T_tm1[:, 0, :], h1T_tm1[:, 1, :]],
                                w1_sb, 2, start=False, stop=True)
                        else:
                            emit_gate_mms(psz1, [], w1_sb, 2,
                                          start=False, stop=True)
                    # L2(t-1): all inputs ready at slot start
                    if t >= 1:
                        tp0 = 32 * (t - 1)
                        psz2 = zp.tile([32, G4], F32, tag="z")
                        lhsTs2 = [h1T_tm1[:, 0, :], h1T_tm1[:, 1, :]]
                        if t >= 2:
                            tq0 = 32 * (t - 2)
                            lhsTs2 += [hs[:, 0, tq0:tq0 + 32],
                                       hs[:, 1, tq0:tq0 + 32]]
                        emit_gate_mms(psz2, lhsTs2, w2_sb, 0,
                                      start=True, stop=True)

                    if psz1 is not None:
                        h1row = lstm_tail(psz1, c1,
                                          b1_sb if has_b1 else None)
                        h1T = lw.tile([128, 2, 32], BF16, tag="h1T")
                        transpose_to(h1row, h1T[:])
                        h1T_prev = h1T
                    # exp fills the ACT gap between the two layer tails
                    if estate is not None:
                        emit_exp(estate)
                    if psz2 is not None:
                        h2row = lstm_tail(psz2, c2,
                                          b2_sb if has_b2 else None)
                        tp0 = 32 * (t - 1)
                        transpose_to(h2row, hs[:, :, tp0:tp0 + 32])

                # trailing logits pairs
                for ei in range(T - 5, RT * NPAIR):
                    emit_exp(emit_logits_mms(ei // 4, ei % 4))

            nc.sync.dma_start(se_d[:], se_sb[:])
            nc.sync.dma_start(tg_d[:], tg_sb[:])

    nc.compile()
    meta = dict(T=T, V=V, n_cores=n_cores, B=B, H=H, VS=VS, BT=BT, RT=RT,
                CH=CH, NCHUNK=NCHUNK, NPAIR=NPAIR)
    return nc, meta


# ---------------- host-side prep / combine ----------------

def prep_inputs(meta, input_data, targets, embedding, W1, b1, W2, b2,
                softmax_w, softmax_b):
    """Build the per-core input maps (numpy)."""
    B, T, V = meta["B"], meta["T"], meta["V"]
    VS, RT, n_cores = meta["VS"], meta["RT"], meta["n_cores"]
    H = meta["H"]
    G4 = 4 * H

    ids_tm = np.ascontiguousarray(
        np.asarray(input_data, np.int64).T).reshape(-1)
    tgt_tm = np.ascontiguousarray(
        np.asarray(targets, np.int64).T).reshape(-1)
    ids_in = ids_tm.astype(np.int32).reshape(RT, 128, 1)

    # W column permutation [i, j, f, o] (TF order) -> [i, o, j, f], with the
    # 0.5 sigmoid input scale folded into the i/o/f columns (the device adds
    # +0.5 to the f columns in psum and does one plain tanh over all gates)
    perm = np.concatenate([
        np.arange(0, H), np.arange(3 * H, 4 * H),
        np.arange(H, 2 * H), np.arange(2 * H, 3 * H)])
    gate_scale = np.concatenate([
        np.full(2 * H, 0.5, np.float32),          # i, o
        np.ones(H, np.float32),                   # j
        np.full(H, 0.5, np.float32)])             # f

    def prep_w(W):
        Wp = (W[:, perm] * gate_scale[None, :]).astype(ml_dtypes.bfloat16)
        return np.ascontiguousarray(Wp.reshape(4, 128, G4))

    w1_in = prep_w(np.asarray(W1, np.float32))
    w2_in = prep_w(np.asarray(W2, np.float32))
    b1p = np.tile((np.asarray(b1, np.float32)[perm]
                   * gate_scale).reshape(1, G4), (32, 1))
    b2p = np.tile((np.asarray(b2, np.float32)[perm]
                   * gate_scale).reshape(1, G4), (32, 1))

    sw = np.asarray(softmax_w, np.float32)                  # [H, V]
    swb = np.asarray(softmax_b, np.float32)

    # vectorized ap_gather index layout: idx i lives at partition i%16,
    # column i//16, replicated per 16-partition group
    rtA = (np.arange(RT) * 128)[:, None, None]
    pA = (np.arange(128) % 16)[None, :, None]
    qA = (np.arange(8) * 16)[None, None, :]
    gat = rtA + qA + pA                                     # [RT, 128, 8]

    maps, masks = [], []
    for c in range(n_cores):
        shard = sw[:, c * VS:(c + 1) * VS].astype(ml_dtypes.bfloat16)
        sw_in = np.ascontiguousarray(shard.reshape(2, 128, VS))
        swi = sw_in.view(np.int16)
        swp_in = np.ascontiguousarray(
            np.stack([swi, swi], axis=-1))                  # [2,128,VS,2]

        tl = tgt_tm - c * VS
        inr = (tl >= 0) & (tl < VS)
        tlc = np.where(inr, tl, 0).astype(np.int16)
        tgi = tlc[gat]                                      # [RT, 128, 8]
        m = dict(ids=ids_in, emb=np.asarray(embedding, np.float32),
                 w1=w1_in, w2=w2_in, sw=sw_in, swp=swp_in, tgi=tgi)
        if np.any(b1p):
            m["b1p"] = b1p
        if np.any(b2p):
            m["b2p"] = b2p
        if np.any(swb):
            m["swbp"] = np.ascontiguousarray(
                np.tile(swb[c * VS:(c + 1) * VS].reshape(1, VS), (128, 1)))
        maps.append(m)
        masks.append(inr.astype(np.float32))
    return maps, masks, ids_tm, tgt_tm


def combine_outputs(meta, results, masks, tgt_tm, softmax_b):
    """results: list of per-core dicts with se_out [128, RT*NPAIR] and
    tg_out [1, BT]. Returns the scalar cost (np.float32)."""
    B, T, BT = meta["B"], meta["T"], meta["BT"]
    RT, NPAIR = meta["RT"], meta["NPAIR"]
    se_all = np.zeros(BT, np.float64)
    tg_all = np.zeros(BT, np.float64)
    for c, r in enumerate(results):
        se = np.asarray(r["se_out"], np.float64)  # [128, RT*NPAIR]
        se = se.reshape(128, RT, NPAIR).sum(-1)   # [128, RT]
        se_all += se.T.reshape(-1)                # row r = rt*128 + p
        tg_all += np.asarray(r["tg_out"], np.float64)[0] * masks[c]
    tg_all += np.asarray(softmax_b, np.float64)[tgt_tm]
    loss = np.log(se_all) - tg_all
    return np.float32(loss.sum() / B / T)


# ---------------- public entry point ----------------

_CACHE = {}
last_exec_time_ns = None
last_trace_path = None


def _get_built(has_b1, has_b2, has_swb):
    key = (has_b1, has_b2, has_swb)
    if key not in _CACHE:
        _CACHE[key] = build_charrnn(T=T, V=V, n_cores=NCORES,
                                    has_b1=has_b1, has_b2=has_b2,
                                    has_swb=has_swb, num_devices=NCORES)
    return _CACHE[key]


def kernel(input_data, targets, embedding, W1, b1, W2, b2,
           softmax_w, softmax_b, _trace=False):
    global last_exec_time_ns, last_trace_path
    has_b1 = bool(np.any(np.asarray(b1)))
    has_b2 = bool(np.any(np.asarray(b2)))
    has_swb = bool(np.any(np.asarray(softmax_b)))
    nc, meta = _get_built(has_b1, has_b2, has_swb)
    maps, masks, ids_tm, tgt_tm = prep_inputs(
        meta, input_data, targets, embedding, W1, b1, W2, b2,
        softmax_w, softmax_b)
    res = run_bass_kernel_spmd(nc, maps, core_ids=list(range(NCORES)),
                               trace=_trace)
    last_exec_time_ns = res.exec_time_ns
    if res.instructions_and_trace is not None:
        last_trace_path = res.instructions_and_trace[1]
    cost = combine_outputs(meta, res.results, masks, tgt_tm, softmax_b)
    return np.asarray(cost, np.float32)


# revision 12
# speedup vs baseline: 1.5581x; 1.5581x over previous
"""Self-contained Trainium2 Bass kernel for the CharRNN problem:
2-layer LSTM (B=32, T=256, H=256) + V=32000 softmax cross-entropy mean loss.

Strategy (8 NeuronCores, SPMD):
  * the LSTM recurrence is replicated on every core (latency-bound)
  * the softmax matmul + exp is sharded over the vocab: each core owns a
    4000-wide shard of softmax_w, computes logits for all 8192 rows against
    its shard, reduces them to per-row sum(exp(logit)) plus the per-row
    target logit; the host combines loss_r = log(sum_c se_r) - tgt_logit_r

Device-side structure (v3 — chain-optimized):
  * wavefront: slot t runs L1 step t and L2 step t-1 so the two layer
    recurrence chains interleave on the engines
  * h transposes via DVE 32x32 StreamTranspose (2 blocks per op,
    cross-partition writes straight into the hidden-major slabs)
  * gate column order [i, o, j, f] with the 0.5 sigmoid input scale folded
    into W on the host; the forget-gate +0.5 bias is added in PSUM by a
    K=1 ones-row matmul -> ONE tanh ACT call per layer-step
  * cell-update add and the target-logit elementwise multiply run on
    GpSimd (the Vector engine is the busiest)
  * exp over PAIRS of 500-wide vocab chunks ([128,2,500] strided AP);
    per slot the logits MMs are emitted FIRST (PE fill work while the
    recurrence chain runs) and the exp ACT call is emitted BETWEEN the two
    layer tails, filling the ACT gap
"""
import numpy as np
import ml_dtypes
import concourse.bass as bass
import concourse.mybir as mybir
import concourse.tile as tile
from concourse import bacc
from concourse.bass_utils import run_bass_kernel_spmd

F32 = mybir.dt.float32
BF16 = mybir.dt.bfloat16
I32 = mybir.dt.int32
I16 = mybir.dt.int16
AF = mybir.ActivationFunctionType
ALU = mybir.AluOpType

B, T, H, V, NCORES = 32, 256, 256, 32000, 8


def build_charrnn(T=256, V=32000, n_cores=8, has_b1=False, has_b2=False,
                  has_swb=False, num_devices=8):
    B, H = 32, 256
    G4 = 4 * H                      # 1024 gate width
    VS = V // n_cores               # vocab shard per core
    BT = B * T
    RT = BT // 128                  # 128-row tiles (4 steps each)
    assert T % 4 == 0 and BT % 128 == 0

    # one psum BANK per matmul chunk (a matmul may not cross a bank)
    CH = max(d for d in range(1, 513) if VS % d == 0)   # 500
    NCHUNK = VS // CH                                    # 8
    NPAIR = NCHUNK // 2                                  # 4 exp calls per tile

    nc = bacc.Bacc("TRN2", target_bir_lowering=False, debug=False,
                   num_devices=num_devices)

    # ---------------- DRAM I/O ----------------
    ids_d = nc.dram_tensor("ids", (RT, 128, 1), I32, kind="ExternalInput")
    emb_d = nc.dram_tensor("emb", (V, H), F32, kind="ExternalInput")
    w1_d = nc.dram_tensor("w1", (4, 128, G4), BF16, kind="ExternalInput")
    w2_d = nc.dram_tensor("w2", (4, 128, G4), BF16, kind="ExternalInput")
    sw_d = nc.dram_tensor("sw", (2, 128, VS), BF16, kind="ExternalInput")
    swp_d = nc.dram_tensor("swp", (2, 128, VS, 2), I16, kind="ExternalInput")
    tgi_d = nc.dram_tensor("tgi", (RT, 128, 8), I16, kind="ExternalInput")
    if has_b1:
        b1_d = nc.dram_tensor("b1p", (32, G4), F32, kind="ExternalInput")
    if has_b2:
        b2_d = nc.dram_tensor("b2p", (32, G4), F32, kind="ExternalInput")
    if has_swb:
        swb_d = nc.dram_tensor("swbp", (128, VS), F32, kind="ExternalInput")
    se_d = nc.dram_tensor("se_out", (128, RT * NPAIR), F32,
                          kind="ExternalOutput")
    tg_d = nc.dram_tensor("tg_out", (1, BT), F32, kind="ExternalOutput")

    with tile.TileContext(nc) as tc:
        with tc.tile_pool(name="persist", bufs=1) as pp:
            # ---- persistent SBUF ----
            w1_sb = pp.tile([128, 4, G4], BF16, tag="w1")
            w2_sb = pp.tile([128, 4, G4], BF16, tag="w2")
            nc.sync.dma_start(w1_sb[:], w1_d[:].rearrange("k p c -> p k c"))
            nc.sync.dma_start(w2_sb[:], w2_d[:].rearrange("k p c -> p k c"))
            sw_sb = pp.tile([128, 2, VS], BF16, tag="sw")
            nc.sync.dma_start(sw_sb[:], sw_d[:].rearrange("k p c -> p k c"))
            swp_sb = pp.tile([128, 2, VS, 2], I16, tag="swp")
            nc.sync.dma_start(swp_sb[:],
                              swp_d[:].rearrange("k p c d -> p k c d"))
            hs = pp.tile([128, 2, BT], BF16, tag="hs")

            ones_bf = pp.tile([128, 1], BF16, tag="ones")
            nc.gpsimd.memset(ones_bf[:], 1.0)
            # forget-gate bias row: z_f += 0.5 (post W-fold) via a K=1
            # matmul closing the half-1 accumulation group
            fb_row = pp.tile([1, 512], BF16, tag="fbrow")
            nc.gpsimd.memset(fb_row[:, 0:256], 0.0)
            nc.gpsimd.memset(fb_row[:, 256:512], 0.5)

            c1 = pp.tile([32, H], F32, tag="c1")
            c2 = pp.tile([32, H], F32, tag="c2")
            nc.gpsimd.memset(c1[:], 0.0)
            nc.gpsimd.memset(c2[:], 0.0)

            se_sb = pp.tile([128, RT * NPAIR], F32, tag="se")
            tg_sb = pp.tile([1, BT], F32, tag="tg")
            # accum_out adds into existing SBUF content on HW — zero it
            nc.gpsimd.memset(se_sb[:], 0.0)

            if has_b1:
                b1_sb = pp.tile([32, G4], F32, tag="b1")
                nc.sync.dma_start(b1_sb[:], b1_d[:])
            if has_b2:
                b2_sb = pp.tile([32, G4], F32, tag="b2")
                nc.sync.dma_start(b2_sb[:], b2_d[:])
            if has_swb:
                swb_sb = pp.tile([128, VS], F32, tag="swb")
                nc.sync.dma_start(swb_sb[:], swb_d[:])

            # ============ fused phase: gather + LSTM + logits ============
            with (
                tc.tile_pool(name="xsp", bufs=1) as xsp,
                tc.tile_pool(name="stage", bufs=3) as stp,
                tc.tile_pool(name="lwork", bufs=3) as lw,
                tc.tile_pool(name="zp", bufs=2, space="PSUM") as zp,
                tc.tile_pool(name="ep", bufs=2, space="PSUM") as ep,
                tc.tile_pool(name="ework", bufs=3) as ew,
            ):
                xs = xsp.tile([128, 2, BT], BF16, tag="xs")

                # ---- embedding gather (time-major) + transpose to slabs ----
                for rt in range(RT):
                    ids_sb = stp.tile([128, 1], I32, tag="ids")
                    nc.gpsimd.dma_start(ids_sb[:], ids_d.ap()[rt])
                    xrow = stp.tile([128, H], F32, tag="xrow")
                    nc.gpsimd.indirect_dma_start(
                        out=xrow[:], out_offset=None,
                        in_=emb_d[:],
                        in_offset=bass.IndirectOffsetOnAxis(
                            ap=ids_sb[:, :1], axis=0),
                    )
                    xbf = stp.tile([128, H], BF16, tag="xbf")
                    nc.vector.tensor_copy(xbf[:], xrow[:])
                    cs = 128 * rt
                    nc.sync.dma_start_transpose(
                        xs[:, 0, cs:cs + 128], xbf[:, 0:128])
                    nc.sync.dma_start_transpose(
                        xs[:, 1, cs:cs + 128], xbf[:, 128:256])

                def emit_logits_mms(rt, p):
                    """Logits matmuls for vocab chunks (2p, 2p+1) of row-tile
                    rt; p==3 also emits the target-logit gather+reduce.
                    Returns state for the deferred exp/copy emission."""
                    cs = 128 * rt
                    pse = ep.tile([128, 2, 512], F32, tag="pse")
                    for half, c in enumerate((2 * p, 2 * p + 1)):
                        for k in range(2):
                            nc.tensor.matmul(
                                pse[:, half, 0:CH], hs[:, k, cs:cs + 128],
                                sw_sb[:, k, c * CH:c * CH + CH],
                                start=(k == 0), stop=(k == 1),
                            )
                        if has_swb:
                            nc.vector.tensor_tensor(
                                out=pse[:, half, 0:CH], in0=pse[:, half, 0:CH],
                                in1=swb_sb[:, (2 * p + half) * CH:
                                           (2 * p + half) * CH + CH],
                                op=ALU.add)
                    pst = None
                    if p == 3:
                        tgi_sb = ew.tile([128, 8], I16, tag="tgi")
                        nc.gpsimd.dma_start(tgi_sb[:], tgi_d.ap()[rt])
                        pstt = ep.tile([128, 2, 512], F32, tag="pse")
                        pst = pstt[0:1, 0, 0:128]
                        for k in range(2):
                            swg = ew.tile([128, 128, 2], I16, tag="swg")
                            nc.gpsimd.ap_gather(
                                swg[:], swp_sb[:, k], tgi_sb[:],
                                channels=128, num_elems=VS, d=2, num_idxs=128,
                            )
                            mulk = ew.tile([128, 128], BF16, tag="mulk")
                            nc.vector.tensor_tensor(
                                out=mulk[:],
                                in0=swg[:].bitcast(BF16)[:, :, 0],
                                in1=hs[:, k, cs:cs + 128],
                                op=ALU.mult)
                            nc.tensor.matmul(pst, ones_bf[:, 0:1], mulk[:],
                                             start=(k == 0), stop=(k == 1))
                    return pse, pst, rt, p, cs

                def emit_exp(state):
                    pse, pst, rt, p, cs = state
                    ebuf = ew.tile([128, 2, CH], BF16, tag="ebuf")
                    nc.scalar.activation(
                        ebuf[:], pse[:, :, 0:CH], AF.Exp,
                        accum_out=se_sb[:, rt * NPAIR + p:rt * NPAIR + p + 1])
                    if pst is not None:
                        nc.scalar.copy(tg_sb[0:1, cs:cs + 128], pst)

                def emit_gate_mms(psz, lhsTs, w_sb, k0, start, stop):
                    """k-tile matmuls into the [32,1024] gate psum; the K=1
                    forget-bias matmul OPENS the half-1 (j,f) group so it is
                    never on the chain-critical tail."""
                    if start:
                        nc.tensor.matmul(
                            psz[:, 512:1024], fb_ones[0:1, :], fb_row[0:1, :],
                            start=True, stop=False)
                    for ki, lt in enumerate(lhsTs):
                        k = k0 + ki
                        first = start and k == 0
                        last = stop and ki == len(lhsTs) - 1
                        nc.tensor.matmul(
                            psz[:, 0:512], lt, w_sb[:, k, 0:512],
                            start=first, stop=last)
                        nc.tensor.matmul(
                            psz[:, 512:1024], lt, w_sb[:, k, 512:1024],
                            start=False, stop=last)

                fb_ones = pp.tile([1, 32], BF16, tag="fbones")
                nc.gpsimd.memset(fb_ones[:], 1.0)

                def lstm_tail(psz, c_sb, bias_sb):
                    """Gate activations + cell update. Gate col order
                    [i, o, j, f]; sigmoid input scales pre-folded into W and
                    the f +0.5 bias added in psum, so ONE plain tanh covers
                    all gates (sigmoid(x) = 0.5*tanh(x/2) + 0.5; the outer
                    affine is applied by affine_mul_reduce)."""
                    if bias_sb is not None:
                        nc.vector.tensor_tensor(
                            out=psz[:], in0=psz[:], in1=bias_sb[:],
                            op=ALU.add)
                    g = lw.tile([32, G4], BF16, tag="g")
                    nc.scalar.activation(g[:], psz[:], AF.Tanh)
                    junk = lw.tile([32, 1], F32, tag="junk")
                    t1 = lw.tile([32, H], F32, tag="t1")
                    nc.vector.affine_mul_reduce(
                        t1[:], junk[:], g[:, 0:256], g[:, 512:768], 0.5, 0.5)
                    cf = lw.tile([32, H], F32, tag="cf")
                    nc.vector.affine_mul_reduce(
                        cf[:], junk[:], g[:, 768:1024], c_sb[:], 0.5, 0.5)
                    nc.vector.tensor_tensor(out=c_sb[:], in0=cf[:],
                                            in1=t1[:], op=ALU.add)
                    tc_t = lw.tile([32, H], BF16, tag="tc")
                    nc.scalar.activation(tc_t[:], c_sb[:], AF.Tanh)
                    hrow = lw.tile([32, H], BF16, tag="hrow")
                    nc.vector.affine_mul_reduce(
                        hrow[:], junk[:], g[:, 256:512], tc_t[:], 0.5, 0.5)
                    return hrow

                def transpose_to(hrow, dst):
                    """hrow [32,256] -> dst [128,2,32] hidden-major k-tiles
                    via DVE StreamTranspose, 2 blocks per op."""
                    hv = hrow[:].rearrange("p (k q b) -> p k q b",
                                           k=2, q=4, b=32)
                    for q in range(4):
                        nc.vector.transpose(
                            dst[32 * q:32 * q + 32], hv[:, :, q, :])

                # ---- wavefront: slot t = L1 step t  +  L2 step t-1 ----
                h1T_prev = None
                for t in range(T + 1):
                    ei = t - 6
                    h1T_tm1 = h1T_prev
                    psz1 = psz2 = None

                    # L1(t) x-part: no dependency on the recurrence
                    if t < T:
                        ts0 = 32 * t
                        psz1 = zp.tile([32, G4], F32, tag="z")
                        emit_gate_mms(
                            psz1,
                            [xs[:, 0, ts0:ts0 + 32], xs[:, 1, ts0:ts0 + 32]],
                            w1_sb, 0, start=True, stop=(h1T_tm1 is None))
                    # PE fill while the chain runs
                    estate = None
                    if ei >= 0:
                        estate = emit_logits_mms(ei // 4, ei % 4)
                    # L1(t) h-part (waits on h1T(t-1))
                    if t < T and h1T_tm1 is not None:
                        emit_gate_mms(
                            psz1, [h1T_tm1[:, 0, :], h1T_tm1[:, 1, :]],
                            w1_sb, 2, start=False, stop=True)
                    # L2(t-1): all inputs ready at slot start
                    if t >= 1:
                        tp0 = 32 * (t - 1)
                        psz2 = zp.tile([32, G4], F32, tag="z")
                        lhsTs2 = [h1T_tm1[:, 0, :], h1T_tm1[:, 1, :]]
                        if t >= 2:
                            tq0 = 32 * (t - 2)
                            lhsTs2 += [hs[:, 0, tq0:tq0 + 32],
                                       hs[:, 1, tq0:tq0 + 32]]
                        emit_gate_mms(psz2, lhsTs2, w2_sb, 0,
                                      start=True, stop=True)

                    if psz1 is not None:
                        h1row = lstm_tail(psz1, c1,
                                          b1_sb if has_b1 else None)
                        h1T = lw.tile([128, 2, 32], BF16, tag="h1T")
                        transpose_to(h1row, h1T[:])
                        h1T_prev = h1T
                    # exp fills the ACT gap between the two layer tails
                    if estate is not None:
                        emit_exp(estate)
                    if psz2 is not None:
                        h2row = lstm_tail(psz2, c2,
                                          b2_sb if has_b2 else None)
                        tp0 = 32 * (t - 1)
                        transpose_to(h2row, hs[:, :, tp0:tp0 + 32])

                # trailing logits pairs
                for ei in range(T - 5, RT * NPAIR):
                    emit_exp(emit_logits_mms(ei // 4, ei % 4))

            nc.sync.dma_start(se_d[:], se_sb[:])
            nc.sync.dma_start(tg_d[:], tg_sb[:])

    nc.compile()
    meta = dict(T=T, V=V, n_cores=n_cores, B=B, H=H, VS=VS, BT=BT, RT=RT,
                CH=CH, NCHUNK=NCHUNK, NPAIR=NPAIR)
    return nc, meta


# ---------------- host-side prep / combine ----------------

def prep_inputs(meta, input_data, targets, embedding, W1, b1, W2, b2,
                softmax_w, softmax_b):
    """Build the per-core input maps (numpy)."""
    B, T, V = meta["B"], meta["T"], meta["V"]
    VS, RT, n_cores = meta["VS"], meta["RT"], meta["n_cores"]
    H = meta["H"]
    G4 = 4 * H

    ids_tm = np.ascontiguousarray(
        np.asarray(input_data, np.int64).T).reshape(-1)
    tgt_tm = np.ascontiguousarray(
        np.asarray(targets, np.int64).T).reshape(-1)
    ids_in = ids_tm.astype(np.int32).reshape(RT, 128, 1)

    # W column permutation [i, j, f, o] (TF order) -> [i, o, j, f], with the
    # 0.5 sigmoid input scale folded into the i/o/f columns (the device adds
    # +0.5 to the f columns in psum and does one plain tanh over all gates)
    perm = np.concatenate([
        np.arange(0, H), np.arange(3 * H, 4 * H),
        np.arange(H, 2 * H), np.arange(2 * H, 3 * H)])
    gate_scale = np.concatenate([
        np.full(2 * H, 0.5, np.float32),          # i, o
        np.ones(H, np.float32),                   # j
        np.full(H, 0.5, np.float32)])             # f

    def prep_w(W):
        Wp = (W[:, perm] * gate_scale[None, :]).astype(ml_dtypes.bfloat16)
        return np.ascontiguousarray(Wp.reshape(4, 128, G4))

    w1_in = prep_w(np.asarray(W1, np.float32))
    w2_in = prep_w(np.asarray(W2, np.float32))
    b1p = np.tile((np.asarray(b1, np.float32)[perm]
                   * gate_scale).reshape(1, G4), (32, 1))
    b2p = np.tile((np.asarray(b2, np.float32)[perm]
                   * gate_scale).reshape(1, G4), (32, 1))

    sw = np.asarray(softmax_w, np.float32)                  # [H, V]
    swb = np.asarray(softmax_b, np.float32)

    # vectorized ap_gather index layout: idx i lives at partition i%16,
    # column i//16, replicated per 16-partition group
    rtA = (np.arange(RT) * 128)[:, None, None]
    pA = (np.arange(128) % 16)[None, :, None]
    qA = (np.arange(8) * 16)[None, None, :]
    gat = rtA + qA + pA                                     # [RT, 128, 8]

    maps, masks = [], []
    for c in range(n_cores):
        shard = sw[:, c * VS:(c + 1) * VS].astype(ml_dtypes.bfloat16)
        sw_in = np.ascontiguousarray(shard.reshape(2, 128, VS))
        swi = sw_in.view(np.int16)
        swp_in = np.ascontiguousarray(
            np.stack([swi, swi], axis=-1))                  # [2,128,VS,2]

        tl = tgt_tm - c * VS
        inr = (tl >= 0) & (tl < VS)
        tlc = np.where(inr, tl, 0).astype(np.int16)
        tgi = tlc[gat]                                      # [RT, 128, 8]
        m = dict(ids=ids_in, emb=np.asarray(embedding, np.float32),
                 w1=w1_in, w2=w2_in, sw=sw_in, swp=swp_in, tgi=tgi)
        if np.any(b1p):
            m["b1p"] = b1p
        if np.any(b2p):
            m["b2p"] = b2p
        if np.any(swb):
            m["swbp"] = np.ascontiguousarray(
                np.tile(swb[c * VS:(c + 1) * VS].reshape(1, VS), (128, 1)))
        maps.append(m)
        masks.append(inr.astype(np.float32))
    return maps, masks, ids_tm, tgt_tm


def combine_outputs(meta, results, masks, tgt_tm, softmax_b):
    """results: list of per-core dicts with se_out [128, RT*NPAIR] and
    tg_out [1, BT]. Returns the scalar cost (np.float32)."""
    B, T, BT = meta["B"], meta["T"], meta["BT"]
    RT, NPAIR = meta["RT"], meta["NPAIR"]
    se_all = np.zeros(BT, np.float64)
    tg_all = np.zeros(BT, np.float64)
    for c, r in enumerate(results):
        se = np.asarray(r["se_out"], np.float64)  # [128, RT*NPAIR]
        se = se.reshape(128, RT, NPAIR).sum(-1)   # [128, RT]
        se_all += se.T.reshape(-1)                # row r = rt*128 + p
        tg_all += np.asarray(r["tg_out"], np.float64)[0] * masks[c]
    tg_all += np.asarray(softmax_b, np.float64)[tgt_tm]
    loss = np.log(se_all) - tg_all
    return np.float32(loss.sum() / B / T)


# ---------------- public entry point ----------------

_CACHE = {}
last_exec_time_ns = None
last_trace_path = None


def _get_built(has_b1, has_b2, has_swb):
    key = (has_b1, has_b2, has_swb)
    if key not in _CACHE:
        _CACHE[key] = build_charrnn(T=T, V=V, n_cores=NCORES,
                                    has_b1=has_b1, has_b2=has_b2,
                                    has_swb=has_swb, num_devices=NCORES)
    return _CACHE[key]


def kernel(input_data, targets, embedding, W1, b1, W2, b2,
           softmax_w, softmax_b, _trace=False):
    global last_exec_time_ns, last_trace_path
    has_b1 = bool(np.any(np.asarray(b1)))
    has_b2 = bool(np.any(np.asarray(b2)))
    has_swb = bool(np.any(np.asarray(softmax_b)))
    nc, meta = _get_built(has_b1, has_b2, has_swb)
    maps, masks, ids_tm, tgt_tm = prep_inputs(
        meta, input_data, targets, embedding, W1, b1, W2, b2,
        softmax_w, softmax_b)
    res = run_bass_kernel_spmd(nc, maps, core_ids=list(range(NCORES)),
                               trace=_trace)
    last_exec_time_ns = res.exec_time_ns
    if res.instructions_and_trace is not None:
        last_trace_path = res.instructions_and_trace[1]
    cost = combine_outputs(meta, res.results, masks, tgt_tm, softmax_b)
    return np.asarray(cost, np.float32)


# revision 15
# speedup vs baseline: 1.5700x; 1.0076x over previous
"""Self-contained Trainium2 Bass kernel for the CharRNN problem:
2-layer LSTM (B=32, T=256, H=256) + V=32000 softmax cross-entropy mean loss.

Strategy (8 NeuronCores, SPMD):
  * the LSTM recurrence is replicated on every core (latency-bound)
  * the softmax matmul + exp is sharded over the vocab: each core owns a
    4000-wide shard of softmax_w, computes logits for all 8192 rows against
    its shard, reduces them to per-row sum(exp(logit)) plus the per-row
    target logit; the host combines loss_r = log(sum_c se_r) - tgt_logit_r

Device-side structure (v3 — chain-optimized):
  * wavefront: slot t runs L1 step t and L2 step t-1 so the two layer
    recurrence chains interleave on the engines
  * h transposes via DVE 32x32 StreamTranspose (2 blocks per op,
    cross-partition writes straight into the hidden-major slabs)
  * gate column order [i, o, j, f] with the 0.5 sigmoid input scale folded
    into W on the host; the forget-gate +0.5 bias is added in PSUM by a
    K=1 ones-row matmul -> ONE tanh ACT call per layer-step
  * cell-update add and the target-logit elementwise multiply run on
    GpSimd (the Vector engine is the busiest)
  * exp over PAIRS of 500-wide vocab chunks ([128,2,500] strided AP);
    per slot the logits MMs are emitted FIRST (PE fill work while the
    recurrence chain runs) and the exp ACT call is emitted BETWEEN the two
    layer tails, filling the ACT gap
"""
import numpy as np
import ml_dtypes
import concourse.bass as bass
import concourse.mybir as mybir
import concourse.tile as tile
from concourse import bacc
from concourse.bass_utils import run_bass_kernel_spmd

F32 = mybir.dt.float32
BF16 = mybir.dt.bfloat16
I32 = mybir.dt.int32
I16 = mybir.dt.int16
AF = mybir.ActivationFunctionType
ALU = mybir.AluOpType

B, T, H, V, NCORES = 32, 256, 256, 32000, 8


def build_charrnn(T=256, V=32000, n_cores=8, has_b1=False, has_b2=False,
                  has_swb=False, num_devices=8):
    B, H = 32, 256
    G4 = 4 * H                      # 1024 gate width
    VS = V // n_cores               # vocab shard per core
    BT = B * T
    RT = BT // 128                  # 128-row tiles (4 steps each)
    assert T % 4 == 0 and BT % 128 == 0

    # one psum BANK per matmul chunk (a matmul may not cross a bank)
    CH = max(d for d in range(1, 513) if VS % d == 0)   # 500
    NCHUNK = VS // CH                                    # 8
    NPAIR = NCHUNK // 2                                  # 4 exp calls per tile

    nc = bacc.Bacc("TRN2", target_bir_lowering=False, debug=False,
                   num_devices=num_devices)

    # ---------------- DRAM I/O ----------------
    ids_d = nc.dram_tensor("ids", (RT, 128, 1), I32, kind="ExternalInput")
    emb_d = nc.dram_tensor("emb", (V, H), F32, kind="ExternalInput")
    w1_d = nc.dram_tensor("w1", (4, 128, G4), BF16, kind="ExternalInput")
    w2_d = nc.dram_tensor("w2", (4, 128, G4), BF16, kind="ExternalInput")
    sw_d = nc.dram_tensor("sw", (2, 128, VS), BF16, kind="ExternalInput")
    swp_d = nc.dram_tensor("swp", (2, 128, VS, 2), I16, kind="ExternalInput")
    tgi_d = nc.dram_tensor("tgi", (RT, 128, 8), I16, kind="ExternalInput")
    if has_b1:
        b1_d = nc.dram_tensor("b1p", (32, G4), F32, kind="ExternalInput")
    if has_b2:
        b2_d = nc.dram_tensor("b2p", (32, G4), F32, kind="ExternalInput")
    if has_swb:
        swb_d = nc.dram_tensor("swbp", (128, VS), F32, kind="ExternalInput")
    se_d = nc.dram_tensor("se_out", (128, RT * NPAIR), F32,
                          kind="ExternalOutput")
    tg_d = nc.dram_tensor("tg_out", (1, BT), F32, kind="ExternalOutput")

    with tile.TileContext(nc) as tc:
        with tc.tile_pool(name="persist", bufs=1) as pp:
            # ---- persistent SBUF ----
            w1_sb = pp.tile([128, 4, G4], BF16, tag="w1")
            w2_sb = pp.tile([128, 4, G4], BF16, tag="w2")
            nc.sync.dma_start(w1_sb[:], w1_d[:].rearrange("k p c -> p k c"))
            nc.sync.dma_start(w2_sb[:], w2_d[:].rearrange("k p c -> p k c"))
            sw_sb = pp.tile([128, 2, VS], BF16, tag="sw")
            nc.sync.dma_start(sw_sb[:], sw_d[:].rearrange("k p c -> p k c"))
            swp_sb = pp.tile([128, 2, VS, 2], I16, tag="swp")
            nc.sync.dma_start(swp_sb[:],
                              swp_d[:].rearrange("k p c d -> p k c d"))
            hs = pp.tile([128, 2, BT], BF16, tag="hs")

            ones_bf = pp.tile([128, 1], BF16, tag="ones")
            nc.gpsimd.memset(ones_bf[:], 1.0)
            # forget-gate bias row: z_f += 0.5 (post W-fold) via a K=1
            # matmul closing the half-1 accumulation group
            fb_row = pp.tile([1, 512], BF16, tag="fbrow")
            nc.gpsimd.memset(fb_row[:, 0:256], 0.0)
            nc.gpsimd.memset(fb_row[:, 256:512], 0.5)

            c1 = pp.tile([32, H], F32, tag="c1")
            c2 = pp.tile([32, H], F32, tag="c2")
            nc.gpsimd.memset(c1[:], 0.0)
            nc.gpsimd.memset(c2[:], 0.0)

            se_sb = pp.tile([128, RT * NPAIR], F32, tag="se")
            tg_sb = pp.tile([1, BT], F32, tag="tg")
            # accum_out adds into existing SBUF content on HW — zero it
            nc.gpsimd.memset(se_sb[:], 0.0)

            if has_b1:
                b1_sb = pp.tile([32, G4], F32, tag="b1")
                nc.sync.dma_start(b1_sb[:], b1_d[:])
            if has_b2:
                b2_sb = pp.tile([32, G4], F32, tag="b2")
                nc.sync.dma_start(b2_sb[:], b2_d[:])
            if has_swb:
                swb_sb = pp.tile([128, VS], F32, tag="swb")
                nc.sync.dma_start(swb_sb[:], swb_d[:])

            # ============ fused phase: gather + LSTM + logits ============
            with (
                tc.tile_pool(name="xsp", bufs=1) as xsp,
                tc.tile_pool(name="stage", bufs=3) as stp,
                tc.tile_pool(name="lwork", bufs=3) as lw,
                tc.tile_pool(name="zp", bufs=2, space="PSUM") as zp,
                tc.tile_pool(name="ep", bufs=2, space="PSUM") as ep,
                tc.tile_pool(name="ework", bufs=3) as ew,
            ):
                xs = xsp.tile([128, 2, BT], BF16, tag="xs")

                # ---- embedding gather (time-major) + transpose to slabs;
                # emitted incrementally from the slot loop so the engine
                # queues are ordered to match data arrival ----
                def emit_gather(rt):
                    ids_sb = stp.tile([128, 1], I32, tag="ids")
                    nc.gpsimd.dma_start(ids_sb[:], ids_d.ap()[rt])
                    xrow = stp.tile([128, H], F32, tag="xrow")
                    nc.gpsimd.indirect_dma_start(
                        out=xrow[:], out_offset=None,
                        in_=emb_d[:],
                        in_offset=bass.IndirectOffsetOnAxis(
                            ap=ids_sb[:, :1], axis=0),
                    )
                    xbf = stp.tile([128, H], BF16, tag="xbf")
                    nc.vector.tensor_copy(xbf[:], xrow[:])
                    cs = 128 * rt
                    nc.sync.dma_start_transpose(
                        xs[:, 0, cs:cs + 128], xbf[:, 0:128])
                    nc.sync.dma_start_transpose(
                        xs[:, 1, cs:cs + 128], xbf[:, 128:256])

                emit_gather(0)
                emit_gather(1)

                def emit_logits_mms(rt, p):
                    """Logits matmuls for vocab chunks (2p, 2p+1) of row-tile
                    rt; p==3 also emits the target-logit gather+reduce.
                    Returns state for the deferred exp/copy emission."""
                    cs = 128 * rt
                    pse = ep.tile([128, 2, 512], F32, tag="pse")
                    for half, c in enumerate((2 * p, 2 * p + 1)):
                        for k in range(2):
                            nc.tensor.matmul(
                                pse[:, half, 0:CH], hs[:, k, cs:cs + 128],
                                sw_sb[:, k, c * CH:c * CH + CH],
                                start=(k == 0), stop=(k == 1),
                            )
                        if has_swb:
                            nc.vector.tensor_tensor(
                                out=pse[:, half, 0:CH], in0=pse[:, half, 0:CH],
                                in1=swb_sb[:, (2 * p + half) * CH:
                                           (2 * p + half) * CH + CH],
                                op=ALU.add)
                    pst = None
                    if p == 3:
                        tgi_sb = ew.tile([128, 8], I16, tag="tgi")
                        nc.gpsimd.dma_start(tgi_sb[:], tgi_d.ap()[rt])
                        pstt = ep.tile([128, 2, 512], F32, tag="pse")
                        pst = pstt[0:1, 0, 0:128]
                        for k in range(2):
                            swg = ew.tile([128, 128, 2], I16, tag="swg")
                            nc.gpsimd.ap_gather(
                                swg[:], swp_sb[:, k], tgi_sb[:],
                                channels=128, num_elems=VS, d=2, num_idxs=128,
                            )
                            mulk = ew.tile([128, 128], BF16, tag="mulk")
                            nc.vector.tensor_tensor(
                                out=mulk[:],
                                in0=swg[:].bitcast(BF16)[:, :, 0],
                                in1=hs[:, k, cs:cs + 128],
                                op=ALU.mult)
                            nc.tensor.matmul(pst, ones_bf[:, 0:1], mulk[:],
                                             start=(k == 0), stop=(k == 1))
                    return pse, pst, rt, p, cs

                def emit_exp(state):
                    pse, pst, rt, p, cs = state
                    ebuf = ew.tile([128, 2, CH], BF16, tag="ebuf")
                    nc.scalar.activation(
                        ebuf[:], pse[:, :, 0:CH], AF.Exp,
                        accum_out=se_sb[:, rt * NPAIR + p:rt * NPAIR + p + 1])
                    if pst is not None:
                        nc.scalar.copy(tg_sb[0:1, cs:cs + 128], pst)

                def emit_gate_mms(psz, lhsTs, w_sb, k0, start, stop):
                    """k-tile matmuls into the [32,1024] gate psum; the K=1
                    forget-bias matmul OPENS the half-1 (j,f) group so it is
                    never on the chain-critical tail."""
                    if start:
                        nc.tensor.matmul(
                            psz[:, 512:1024], fb_ones[0:1, :], fb_row[0:1, :],
                            start=True, stop=False)
                    for ki, lt in enumerate(lhsTs):
                        k = k0 + ki
                        first = start and k == 0
                        last = stop and ki == len(lhsTs) - 1
                        nc.tensor.matmul(
                            psz[:, 0:512], lt, w_sb[:, k, 0:512],
                            start=first, stop=last)
                        nc.tensor.matmul(
                            psz[:, 512:1024], lt, w_sb[:, k, 512:1024],
                            start=False, stop=last)

                fb_ones = pp.tile([1, 32], BF16, tag="fbones")
                nc.gpsimd.memset(fb_ones[:], 1.0)

                def lstm_act(psz, bias_sb):
                    """Gate activation. Gate col order [i, o, j, f]; sigmoid
                    input scales pre-folded into W and the f +0.5 bias added
                    in psum, so ONE plain tanh covers all gates
                    (sigmoid(x) = 0.5*tanh(x/2) + 0.5; the outer affine is
                    applied by affine_mul_reduce)."""
                    if bias_sb is not None:
                        nc.vector.tensor_tensor(
                            out=psz[:], in0=psz[:], in1=bias_sb[:],
                            op=ALU.add)
                    g = lw.tile([32, G4], BF16, tag="g")
                    nc.scalar.activation(g[:], psz[:], AF.Tanh)
                    return g

                def lstm_cell(g, c_sb):
                    junk = lw.tile([32, 1], F32, tag="junk")
                    t1 = lw.tile([32, H], F32, tag="t1")
                    nc.vector.affine_mul_reduce(
                        t1[:], junk[:], g[:, 0:256], g[:, 512:768], 0.5, 0.5)
                    cf = lw.tile([32, H], F32, tag="cf")
                    nc.vector.affine_mul_reduce(
                        cf[:], junk[:], g[:, 768:1024], c_sb[:], 0.5, 0.5)
                    nc.vector.tensor_tensor(out=c_sb[:], in0=cf[:],
                                            in1=t1[:], op=ALU.add)
                    tc_t = lw.tile([32, H], BF16, tag="tc")
                    nc.scalar.activation(tc_t[:], c_sb[:], AF.Tanh)
                    hrow = lw.tile([32, H], BF16, tag="hrow")
                    nc.vector.affine_mul_reduce(
                        hrow[:], junk[:], g[:, 256:512], tc_t[:], 0.5, 0.5)
                    return hrow

                def transpose_to(hrow, dst):
                    """hrow [32,256] -> dst [128,2,32] hidden-major k-tiles
                    via DVE StreamTranspose, 2 blocks per op."""
                    hv = hrow[:].rearrange("p (k q b) -> p k q b",
                                           k=2, q=4, b=32)
                    for q in range(4):
                        nc.vector.transpose(
                            dst[32 * q:32 * q + 32], hv[:, :, q, :])

                # ---- wavefront: slot t = L1 step t  +  L2 step t-1 ----
                h1T_prev = None
                for t in range(T + 1):
                    ei = t - 6
                    h1T_tm1 = h1T_prev
                    psz1 = psz2 = None

                    # L1(t) x-part: no dependency on the recurrence
                    if t < T:
                        ts0 = 32 * t
                        psz1 = zp.tile([32, G4], F32, tag="z")
                        emit_gate_mms(
                            psz1,
                            [xs[:, 0, ts0:ts0 + 32], xs[:, 1, ts0:ts0 + 32]],
                            w1_sb, 0, start=True, stop=(h1T_tm1 is None))
                    # PE fill while the chain runs
                    estate = None
                    if ei >= 0:
                        estate = emit_logits_mms(ei // 4, ei % 4)
                    # L1(t) h-part (waits on h1T(t-1))
                    if t < T and h1T_tm1 is not None:
                        emit_gate_mms(
                            psz1, [h1T_tm1[:, 0, :], h1T_tm1[:, 1, :]],
                            w1_sb, 2, start=False, stop=True)
                    # L2(t-1): all inputs ready at slot start
                    if t >= 1:
                        tp0 = 32 * (t - 1)
                        psz2 = zp.tile([32, G4], F32, tag="z")
                        lhsTs2 = [h1T_tm1[:, 0, :], h1T_tm1[:, 1, :]]
                        if t >= 2:
                            tq0 = 32 * (t - 2)
                            lhsTs2 += [hs[:, 0, tq0:tq0 + 32],
                                       hs[:, 1, tq0:tq0 + 32]]
                        emit_gate_mms(psz2, lhsTs2, w2_sb, 0,
                                      start=True, stop=True)

                    # incremental embedding gather, 2 tiles ahead
                    if t % 4 == 0 and t // 4 + 2 < RT:
                        emit_gather(t // 4 + 2)

                    # both gate ACTs first (the second runs while the first
                    # layer's DVE cell chain executes), then the cell chains
                    g1 = lstm_act(psz1, b1_sb if has_b1 else None) \
                        if psz1 is not None else None
                    g2 = lstm_act(psz2, b2_sb if has_b2 else None) \
                        if psz2 is not None else None
                    if g1 is not None:
                        h1row = lstm_cell(g1, c1)
                        h1T = lw.tile([128, 2, 32], BF16, tag="h1T")
                        transpose_to(h1row, h1T[:])
                        h1T_prev = h1T
                    if g2 is not None:
                        h2row = lstm_cell(g2, c2)
                        tp0 = 32 * (t - 1)
                        transpose_to(h2row, hs[:, :, tp0:tp0 + 32])
                    # exp at slot end: overlaps the next slot's MM phase
                    if estate is not None:
                        emit_exp(estate)

                # trailing logits pairs
                for ei in range(T - 5, RT * NPAIR):
                    emit_exp(emit_logits_mms(ei // 4, ei % 4))

            nc.sync.dma_start(se_d[:], se_sb[:])
            nc.sync.dma_start(tg_d[:], tg_sb[:])

    nc.compile()
    meta = dict(T=T, V=V, n_cores=n_cores, B=B, H=H, VS=VS, BT=BT, RT=RT,
                CH=CH, NCHUNK=NCHUNK, NPAIR=NPAIR)
    return nc, meta


# ---------------- host-side prep / combine ----------------

def prep_inputs(meta, input_data, targets, embedding, W1, b1, W2, b2,
                softmax_w, softmax_b):
    """Build the per-core input maps (numpy)."""
    B, T, V = meta["B"], meta["T"], meta["V"]
    VS, RT, n_cores = meta["VS"], meta["RT"], meta["n_cores"]
    H = meta["H"]
    G4 = 4 * H

    ids_tm = np.ascontiguousarray(
        np.asarray(input_data, np.int64).T).reshape(-1)
    tgt_tm = np.ascontiguousarray(
        np.asarray(targets, np.int64).T).reshape(-1)
    ids_in = ids_tm.astype(np.int32).reshape(RT, 128, 1)

    # W column permutation [i, j, f, o] (TF order) -> [i, o, j, f], with the
    # 0.5 sigmoid input scale folded into the i/o/f columns (the device adds
    # +0.5 to the f columns in psum and does one plain tanh over all gates)
    perm = np.concatenate([
        np.arange(0, H), np.arange(3 * H, 4 * H),
        np.arange(H, 2 * H), np.arange(2 * H, 3 * H)])
    gate_scale = np.concatenate([
        np.full(2 * H, 0.5, np.float32),          # i, o
        np.ones(H, np.float32),                   # j
        np.full(H, 0.5, np.float32)])             # f

    def prep_w(W):
        Wp = (W[:, perm] * gate_scale[None, :]).astype(ml_dtypes.bfloat16)
        return np.ascontiguousarray(Wp.reshape(4, 128, G4))

    w1_in = prep_w(np.asarray(W1, np.float32))
    w2_in = prep_w(np.asarray(W2, np.float32))
    b1p = np.tile((np.asarray(b1, np.float32)[perm]
                   * gate_scale).reshape(1, G4), (32, 1))
    b2p = np.tile((np.asarray(b2, np.float32)[perm]
                   * gate_scale).reshape(1, G4), (32, 1))

    sw = np.asarray(softmax_w, np.float32)                  # [H, V]
    swb = np.asarray(softmax_b, np.float32)

    # vectorized ap_gather index layout: idx i lives at partition i%16,
    # column i//16, replicated per 16-partition group
    rtA = (np.arange(RT) * 128)[:, None, None]
    pA = (np.arange(128) % 16)[None, :, None]
    qA = (np.arange(8) * 16)[None, None, :]
    gat = rtA + qA + pA                                     # [RT, 128, 8]

    maps, masks = [], []
    for c in range(n_cores):
        shard = sw[:, c * VS:(c + 1) * VS].astype(ml_dtypes.bfloat16)
        sw_in = np.ascontiguousarray(shard.reshape(2, 128, VS))
        swi = sw_in.view(np.int16)
        swp_in = np.ascontiguousarray(
            np.stack([swi, swi], axis=-1))                  # [2,128,VS,2]

        tl = tgt_tm - c * VS
        inr = (tl >= 0) & (tl < VS)
        tlc = np.where(inr, tl, 0).astype(np.int16)
        tgi = tlc[gat]                                      # [RT, 128, 8]
        m = dict(ids=ids_in, emb=np.asarray(embedding, np.float32),
                 w1=w1_in, w2=w2_in, sw=sw_in, swp=swp_in, tgi=tgi)
        if np.any(b1p):
            m["b1p"] = b1p
        if np.any(b2p):
            m["b2p"] = b2p
        if np.any(swb):
            m["swbp"] = np.ascontiguousarray(
                np.tile(swb[c * VS:(c + 1) * VS].reshape(1, VS), (128, 1)))
        maps.append(m)
        masks.append(inr.astype(np.float32))
    return maps, masks, ids_tm, tgt_tm


def combine_outputs(meta, results, masks, tgt_tm, softmax_b):
    """results: list of per-core dicts with se_out [128, RT*NPAIR] and
    tg_out [1, BT]. Returns the scalar cost (np.float32)."""
    B, T, BT = meta["B"], meta["T"], meta["BT"]
    RT, NPAIR = meta["RT"], meta["NPAIR"]
    se_all = np.zeros(BT, np.float64)
    tg_all = np.zeros(BT, np.float64)
    for c, r in enumerate(results):
        se = np.asarray(r["se_out"], np.float64)  # [128, RT*NPAIR]
        se = se.reshape(128, RT, NPAIR).sum(-1)   # [128, RT]
        se_all += se.T.reshape(-1)                # row r = rt*128 + p
        tg_all += np.asarray(r["tg_out"], np.float64)[0] * masks[c]
    tg_all += np.asarray(softmax_b, np.float64)[tgt_tm]
    loss = np.log(se_all) - tg_all
    return np.float32(loss.sum() / B / T)


# ---------------- public entry point ----------------

_CACHE = {}
last_exec_time_ns = None
last_trace_path = None


def _get_built(has_b1, has_b2, has_swb):
    key = (has_b1, has_b2, has_swb)
    if key not in _CACHE:
        _CACHE[key] = build_charrnn(T=T, V=V, n_cores=NCORES,
                                    has_b1=has_b1, has_b2=has_b2,
                                    has_swb=has_swb, num_devices=NCORES)
    return _CACHE[key]


def kernel(input_data, targets, embedding, W1, b1, W2, b2,
           softmax_w, softmax_b, _trace=False):
    global last_exec_time_ns, last_trace_path
    has_b1 = bool(np.any(np.asarray(b1)))
    has_b2 = bool(np.any(np.asarray(b2)))
    has_swb = bool(np.any(np.asarray(softmax_b)))
    nc, meta = _get_built(has_b1, has_b2, has_swb)
    maps, masks, ids_tm, tgt_tm = prep_inputs(
        meta, input_data, targets, embedding, W1, b1, W2, b2,
        softmax_w, softmax_b)
    res = run_bass_kernel_spmd(nc, maps, core_ids=list(range(NCORES)),
                               trace=_trace)
    last_exec_time_ns = res.exec_time_ns
    if res.instructions_and_trace is not None:
        last_trace_path = res.instructions_and_trace[1]
    cost = combine_outputs(meta, res.results, masks, tgt_tm, softmax_b)
    return np.asarray(cost, np.float32)
